# revision 28
# baseline (speedup 1.0000x reference)
"""Trainium2 Bass kernel for nn_AdaptiveBlock (B=64,T=512,H=1024,K=49).

Data-parallel over batch: 8 NeuronCores x 8 examples each, weights replicated.

Math (per example; h0=0 so the Whh term vanishes):
  th_g = tanh(0.5 * x @ Wx.T)            # sigmoid(z) = 0.5*(1+tanh(z/2))
  s2   = (1 + th_g) * tanh(cells)        # s = 0.5*s2
  g    = hiddens @ Wg.T                  # [T,49]
  cv   = V @ Wv.T                        # [49,49]
  z_t[t,k]  = sum_j Wh[j] tanh(cv[k,j] + g[t,j])
  z_ext[t]  = sum_j Wh[j] tanh((s2 @ (Ws/2).T + g)[t,j])
  e = exp([z_t, z_ext]); w = e/sum(e); beta' = 0.5*e_ext/sum(e)   # |z|<=6
  out = w[:, :49] @ V + beta' * s2 + hiddens

Host does all layout work (bf16/fp8 casts, transposes to [H,T], weight
transposes, selector/broadcast constants); device computes in [h-part,
t-free] layout and writes out^T [H,T] bf16, un-transposed on host.
Content chain uses the ecv selector-matmul trick: one matmul per psum
piece materializes cv[k,j]+g[t,j] for all (k,j). The chain runs as two
concurrent PE streams (row groups 0-63 / 64-127, duplicated gTx/ecv
rows) covering two t-chunks at once. Gate matmul runs fp8 DoubleRow.
Three-deep software pipeline interleaved at t-chunk grain:
epilogue(b-1) | chain+softmax(b) | gate(b+1).
"""

import os
import numpy as np

import concourse.bass as bass
import concourse.mybir as mybir
from concourse import bacc
import concourse.tile as tile
from concourse.masks import make_identity

FP32 = mybir.dt.float32
BF16 = mybir.dt.bfloat16
FP8 = mybir.dt.float8e4
AX = mybir.AxisListType
OP = mybir.AluOpType
AF = mybir.ActivationFunctionType

B, T, H, K = 64, 512, 1024, 49
NC_ = 8             # cores
BPC = B // NC_      # examples per core
TC = T // 128       # 4 t-chunks
HC = H // 128       # 8 h-chunks
# chain layout: three bank-aligned-enough pieces, segment = 49 cols (no pad)
PIECES = ((0, 0, 20), (980, 20, 20), (1960, 40, 9))  # (col, seg0, nsegs)
CW = 2401           # 49 segments x 49 j
CWE = 2402          # even tile width: keeps pool buffers 4B-aligned
MG = 114            # g-matmul output rows: 0-49 = g|ones, 64-113 = dup


def build(nc: bass.Bass):
    xT_d = nc.declare_dram_parameter("xT", [BPC, 128, 4096], FP8, isOutput=False)
    hT_d = nc.declare_dram_parameter("hT", [BPC, H, T], BF16, isOutput=False)
    cT_d = nc.declare_dram_parameter("cT", [BPC, H, T], BF16, isOutput=False)
    v_d = nc.declare_dram_parameter("vp", [BPC, 64, H], BF16, isOutput=False)
    vT_d = nc.declare_dram_parameter("vT", [BPC, H, 64], BF16, isOutput=False)
    wxT_d = nc.declare_dram_parameter("wxT", [128, 8192], FP8, isOutput=False)
    h8_d = nc.declare_dram_parameter("h8", [BPC, 128, 4096], FP8, isOutput=False)
    wgT_d = nc.declare_dram_parameter("wg8", [128, 1024], FP8, isOutput=False)
    wsT_d = nc.declare_dram_parameter("wsT2", [H, 64], BF16, isOutput=False)
    wvT_d = nc.declare_dram_parameter("wvT", [H, 64], BF16, isOutput=False)
    whT_d = nc.declare_dram_parameter("whT", [64, 1], BF16, isOutput=False)
    whf_d = nc.declare_dram_parameter("whf", [128, CW], BF16, isOutput=False)
    ecv_d = nc.declare_dram_parameter("ecvb", [128, CW], BF16, isOutput=False)
    out_d = nc.declare_dram_parameter("out", [BPC, H, T], BF16, isOutput=True)

    with tile.TileContext(nc) as tc:
        with (
            tc.tile_pool(name="const", bufs=1) as constp,
            tc.tile_pool(name="inp", bufs=2) as inp,
            tc.tile_pool(name="act", bufs=2) as actp,
            tc.tile_pool(name="sm", bufs=2) as smp,
            tc.tile_pool(name="outp", bufs=2) as outp,
            tc.tile_pool(name="psA", bufs=2, space="PSUM") as psA,
            tc.tile_pool(name="psB", bufs=2, space="PSUM") as psB,
            tc.tile_pool(name="psC", bufs=2, space="PSUM") as psC,
        ):
            # ---------------- constants / tiny memsets ----------
            ident = constp.tile([128, 128], BF16, name="ident")
            make_identity(nc, ident)
            ident_f = constp.tile([128, 128], FP32, name="ident_f")
            make_identity(nc, ident_f)
            ident32 = constp.tile([128, 128], BF16, name="ident32")
            nc.vector.tensor_scalar_mul(ident32, ident, 1.0 / 32.0)
            ones1 = constp.tile([1, 128], BF16, name="ones1")
            nc.gpsimd.memset(ones1, 1.0)
            onesT = constp.tile([1, T], BF16, name="onesT")
            nc.gpsimd.memset(onesT, 1.0)
            e49row = constp.tile([1, MG], BF16, name="e49row")
            nc.gpsimd.memset(e49row, 0.0)
            nc.gpsimd.memset(e49row[0:1, K:K + 1], 1.0)
            nc.gpsimd.memset(e49row[0:1, MG - 1:MG], 1.0)

            state = {}

            # ---------------- input loads (gate-critical first) ----------
            def loads_gate(b):
                xT = inp.tile([128, 4096], FP8, tag="xT", bufs=3, name=f"xT{b}")
                nc.sync.dma_start(out=xT, in_=xT_d[b])
                cT = inp.tile([128, HC * T], BF16, tag="cT", bufs=2, name=f"cT{b}")
                nc.gpsimd.dma_start(
                    out=cT.rearrange("p (c t) -> p c t", t=T),
                    in_=cT_d[b].rearrange("(c p) t -> p c t", p=128),
                )
                state[b] = {"xT": xT, "cT": cT}

            def loads_rest(b):
                st = state[b]
                hT = inp.tile([128, HC * T], BF16, tag="hT", bufs=4, name=f"hT{b}")
                nc.gpsimd.dma_start(
                    out=hT.rearrange("p (c t) -> p c t", t=T),
                    in_=hT_d[b].rearrange("(c p) t -> p c t", p=128),
                )
                h8 = inp.tile([128, 4096], FP8, tag="h8", bufs=3, name=f"h8{b}")
                nc.sync.dma_start(out=h8, in_=h8_d[b])
                st["h8"] = h8
                vn = inp.tile([64, H], BF16, tag="vn", bufs=4, name=f"vn{b}")
                nc.sync.dma_start(out=vn, in_=v_d[b])
                vT = inp.tile([128, HC * 64], BF16, tag="vT", bufs=3, name=f"vT{b}")
                nc.sync.dma_start(
                    out=vT.rearrange("p (c j) -> p c j", j=64),
                    in_=vT_d[b].rearrange("(c p) j -> p c j", p=128),
                )
                st.update({"hT": hT, "vn": vn, "vT": vT})

            # tanh(cells) for the whole example in one ACT instruction
            def tclf(b):
                st = state[b]
                tcl = actp.tile([128, HC * T], BF16, tag="tclf", bufs=2,
                                name=f"tclf{b}")
                nc.scalar.activation(tcl, st["cT"], AF.Tanh)
                st["tclf"] = tcl

            # ---------------- pipeline stage bodies ----------------
            def head(b):
                st = state[b]
                # cv = V @ Wv.T  -> scatter into ecv rows 49 and 113
                cvp = psA.tile([128, 512], FP32, tag="ps", name=f"cvp{b}")
                for kc in range(HC):
                    nc.tensor.matmul(
                        cvp[0:64, 0:64],
                        st["vT"][:, kc * 64:(kc + 1) * 64],
                        wvT[:, kc * 64:(kc + 1) * 64],
                        start=(kc == 0), stop=(kc == HC - 1),
                    )
                cv_sb = smp.tile([K, K], BF16, tag="cv", name=f"cv{b}")
                nc.vector.tensor_copy(cv_sb, cvp[0:K, 0:K])
                ecv_b = ecv[b % 2]
                for row in (K, 64 + K):
                    for col, s0, ns in PIECES:
                        nc.gpsimd.dma_start(
                            out=ecv_b[row:row + 1, col:col + ns * K].rearrange(
                                "p (s j) -> p s j", j=K),
                            in_=cv_sb[s0:s0 + ns],
                        )
                st["ecv"] = ecv_b
                # gT = 32*(hiddens @ Wg.T).T via fp8 DoubleRow;
                # rows 0-48 g, 49 ones, 64-112 g, 113 ones (1/32 in ecv/ident32)
                gp = psA.tile([128, 512], FP32, tag="ps", name=f"gp{b}")
                wg4 = wg8.rearrange("p (i r o) -> p i r o", r=2, o=128)
                h4 = st["h8"].rearrange("p (i r t) -> p i r t", r=2, t=T)
                for i in range(4):
                    nc.tensor.matmul(
                        gp[0:MG, 0:T],
                        wg4[:, i, :, 0:MG],
                        h4[:, i],
                        start=(i == 0), stop=False,
                        perf_mode=mybir.MatmulPerfMode.DoubleRow,
                    )
                nc.tensor.matmul(
                    gp[0:MG, 0:T], e49row, onesT, start=False, stop=True)
                gTx = smp.tile([128, T], BF16, tag="gTx", name=f"gTx{b}")
                nc.vector.tensor_copy(gTx[0:MG], gp[0:MG, 0:T])
                st["gTx"] = gTx

            def gate_pair(b, pr):
                # hc pair (2*pr, 2*pr+1): fp8 DoubleRow matmuls + tanh + s2
                st = state[b]
                if pr == 0:
                    st["s2"] = actp.tile([128, HC * T], BF16, tag="s2", bufs=3,
                                         name=f"s2{b}")
                wx4 = wxT.rearrange("p (i r o) -> p i r o", r=2, o=H)
                x4 = st["xT"].rearrange("p (i r t) -> p i r t", r=2, t=T)
                for i_ in range(2):
                    hc = 2 * pr + i_
                    pg = psA.tile([128, 512], FP32, tag="ps", name=f"pg{b}_{hc}")
                    for i in range(4):
                        nc.tensor.matmul(
                            pg,
                            wx4[:, i, :, hc * 128:(hc + 1) * 128],
                            x4[:, i],
                            start=(i == 0), stop=(i == 3),
                            perf_mode=mybir.MatmulPerfMode.DoubleRow,
                        )
                    tg = actp.tile([128, T], BF16, tag="thg", bufs=2,
                                   name=f"thg{b}_{hc}")
                    nc.scalar.activation(tg, pg, AF.Tanh, scale=1.0 / 64.0)
                    # s2 = (th_g + 1) * th_c
                    nc.vector.scalar_tensor_tensor(
                        out=st["s2"][:, hc * T:(hc + 1) * T],
                        in0=tg, scalar=1.0,
                        in1=st["tclf"][:, hc * T:(hc + 1) * T],
                        op0=OP.add, op1=OP.mult,
                    )

            def chain_pair(b, h_):
                # two t-chunks (2h, 2h+1) as concurrent PE row-group streams
                st = state[b]
                gTx, ecv_b = st["gTx"], st["ecv"]
                tcbs = (2 * h_, 2 * h_ + 1)
                lhs = (gTx[0:K + 1, tcbs[0] * 128:(tcbs[0] + 1) * 128],
                       gTx[64:64 + K + 1, tcbs[1] * 128:(tcbs[1] + 1) * 128])
                conts = []
                for s_ in range(2):
                    conts.append(smp.tile([128, CWE], BF16, tag="cont",
                                          name=f"cont{b}_{tcbs[s_]}"))
                pps = []
                for col, s0, ns in PIECES:
                    w_ = ns * K
                    pool_ = psC if ns == 9 else psB
                    pp = [pool_.tile([128, 1024 if w_ > 512 else 512], FP32,
                                     tag="pb", name=f"pp{b}_{tcbs[s_]}_{col}")
                          for s_ in range(2)]
                    off = 0
                    while off < w_:
                        cw = min(512, w_ - off)
                        for s_ in range(2):
                            rb = 64 * s_
                            nc.tensor.matmul(
                                pp[s_][:, off:off + cw],
                                lhs[s_],
                                ecv_b[rb:rb + K + 1, col + off:col + off + cw],
                                start=True, stop=True,
                            )
                        off += cw
                    pps.append((pp, col, w_))
                for pp, col, w_ in pps:
                    for s_ in range(2):
                        nc.scalar.activation(
                            conts[s_][:, col:col + w_], pp[s_][:, 0:w_], AF.Tanh)
                for s_ in range(2):
                    cw_t = smp.tile([128, CWE], BF16, tag="cw", bufs=1,
                                    name=f"cw{b}_{tcbs[s_]}")
                    nc.vector.tensor_mul(cw_t, conts[s_], wh_full)
                    nc.vector.tensor_reduce(
                        out=st["zf"][:, tcbs[s_] * 50:tcbs[s_] * 50 + K],
                        in_=cw_t[:, 0:CW].rearrange("p (s j) -> p s j", j=K),
                        axis=AX.X, op=OP.add,
                    )

            def content_s(b):
                st = state[b]
                csp = psA.tile([128, 512], FP32, tag="ps", name=f"csp{b}")
                for kc in range(HC):
                    nc.tensor.matmul(
                        csp[0:64, 0:T],
                        wsT[:, kc * 64:(kc + 1) * 64],
                        st["s2"][:, kc * T:(kc + 1) * T],
                        start=(kc == 0), stop=False,
                    )
                nc.tensor.matmul(
                    csp[0:64, 0:T], ident32[0:K + 1, 0:64], st["gTx"][0:K + 1],
                    start=False, stop=True,
                )
                tcs = smp.tile([K, T], BF16, tag="tcs", name=f"tcs{b}")
                nc.scalar.activation(tcs, csp[0:K, 0:T], AF.Tanh)
                zxp = psA.tile([128, 512], FP32, tag="ps", name=f"zxp{b}")
                for tcb in range(TC):
                    nc.tensor.matmul(
                        zxp[:, tcb:tcb + 1],
                        tcs[:, tcb * 128:(tcb + 1) * 128],
                        whT[0:K],
                        start=True, stop=True,
                        skip_group_check=True,
                    )
                nc.vector.tensor_copy(
                    st["zf"].rearrange("p (c j) -> p c j", j=50)[:, :, K:K + 1],
                    zxp[:, 0:TC])

            def softmax(b):
                # |z| <= sum|Wh| ~ 6 so exp cannot overflow: skip max-subtract
                st = state[b]
                ef = smp.tile([128, 4 * 50], FP32, tag="ef", name=f"ef{b}")
                nc.scalar.activation(ef, st["zf"], AF.Exp)
                den = smp.tile([128, 4], FP32, tag="den", name=f"dn{b}")
                nc.vector.tensor_reduce(
                    out=den, in_=ef.rearrange("p (c j) -> p c j", j=50),
                    axis=AX.X, op=OP.add)
                rec = smp.tile([128, 4], FP32, tag="rec", name=f"rc{b}")
                nc.vector.reciprocal(rec, den)
                brow0 = smp.tile([1, T], BF16, tag="br0", name=f"br0{b}")
                st["brow0"] = brow0
                st["wtp"] = []
                for tcb in range(TC):
                    wb = smp.tile([128, K], FP32, tag="wb", bufs=8, name=f"wb{b}_{tcb}")
                    nc.vector.tensor_scalar_mul(
                        wb, ef[:, tcb * 50:tcb * 50 + K], rec[:, tcb:tcb + 1])
                    bc = smp.tile([128, 1], FP32, tag="bc", bufs=4, name=f"bc{b}_{tcb}")
                    nc.vector.tensor_scalar(
                        out=bc, in0=ef[:, tcb * 50 + K:tcb * 50 + K + 1],
                        scalar1=rec[:, tcb:tcb + 1], scalar2=0.5,
                        op0=OP.mult, op1=OP.mult,
                    )
                    nc.gpsimd.dma_start(
                        out=brow0[0:1, tcb * 128:(tcb + 1) * 128], in_=bc)
                    st["wtp"].append(wb)

            def wT_fin(b):
                st = state[b]
                wTb = smp.tile([K, T], BF16, tag="wT", name=f"wT{b}")
                st["wT"] = wTb
                for tcb in range(TC):
                    wp = psC.tile([128, 512], FP32, tag="pb", name=f"wp{b}_{tcb}")
                    nc.tensor.transpose(
                        wp[0:K, 0:128], st["wtp"][tcb], ident_f)
                    nc.vector.tensor_copy(
                        wTb[:, tcb * 128:(tcb + 1) * 128], wp[0:K, 0:128])
                brp = psC.tile([128, 512], FP32, tag="pb", name=f"brp{b}")
                nc.tensor.matmul(brp, ones1, st["brow0"], start=True, stop=True)
                brow = smp.tile([128, T], BF16, tag="brow", name=f"brow{b}")
                nc.vector.tensor_copy(brow, brp)
                st["brow"] = brow
                st["ob"] = outp.tile([128, HC * T], BF16, tag="ob", name=f"ob{b}")

            def epilogue_pair(b, pr):
                st = state[b]
                cp = psB.tile([128, 1024], FP32, tag="pb", name=f"cp{b}_{pr}")
                ut = smp.tile([128, 1024], BF16, tag="ut", bufs=2, name=f"ut{b}_{pr}")
                nc.gpsimd.tensor_mul(
                    ut.rearrange("p (c t) -> p c t", t=T),
                    st["s2"][:, pr * 1024:(pr + 1) * 1024].rearrange(
                        "p (c t) -> p c t", t=T),
                    st["brow"][:, None, :].broadcast_to([128, 2, T]),
                )
                for i in range(2):
                    hc = 2 * pr + i
                    sl = cp[:, i * T:(i + 1) * T]
                    nc.tensor.matmul(
                        sl,
                        st["vn"][0:K, hc * 128:(hc + 1) * 128],
                        st["wT"],
                        start=True, stop=False,
                    )
                    nc.tensor.matmul(
                        sl, ident,
                        st["hT"][:, hc * T:(hc + 1) * T],
                        start=False, stop=False,
                    )
                    nc.tensor.matmul(
                        sl, ident, ut[:, i * T:(i + 1) * T],
                        start=False, stop=True,
                    )
                nc.scalar.copy(out=st["ob"][:, pr * 1024:(pr + 1) * 1024], in_=cp)

            def out_dma(b):
                st = state[b]
                eng = nc.sync if b % 2 == 0 else nc.scalar
                eng.dma_start(
                    out=out_d[b].rearrange("(c p) t -> p c t", p=128),
                    in_=st["ob"].rearrange("p (c t) -> p c t", t=T),
                )

            def zf_alloc(b):
                state[b]["zf"] = smp.tile([128, 4 * 50], FP32, tag="zf",
                                          name=f"zf{b}")

            # ---------------- prologue: gate(0) before anything else -------
            loads_gate(0)
            wxT = constp.tile([128, 8192], FP8, name="wxT")
            nc.sync.dma_start(out=wxT, in_=wxT_d[:, :])
            tclf(0)
            for pr in range(4):
                gate_pair(0, pr)

            # remaining constants on the scalar queue
            def load_w(dram, name, jw):
                t_ = constp.tile([128, HC * jw], BF16, name=name)
                nc.scalar.dma_start(
                    out=t_.rearrange("p (kc j) -> p kc j", j=jw),
                    in_=dram[:, :].rearrange("(kc p) j -> p kc j", p=128),
                )
                return t_
            wg8 = constp.tile([128, 1024], FP8, name="wg8")
            nc.scalar.dma_start(out=wg8, in_=wgT_d[:, :])
            wsT = load_w(wsT_d, "wsT", 64)
            wvT = load_w(wvT_d, "wvT", 64)
            wh_full = constp.tile([128, CWE], BF16, name="whf")
            nc.scalar.dma_start(out=wh_full[:, 0:CW], in_=whf_d[:, :])
            nc.gpsimd.memset(wh_full[:, CW:CWE], 0.0)
            ecv = []
            for v_ in range(2):
                e_ = constp.tile([128, CWE], BF16, name=f"ecv{v_}")
                nc.scalar.dma_start(out=e_[:, 0:CW], in_=ecv_d[:, :])
                ecv.append(e_)

            loads_rest(0)
            loads_gate(1)
            loads_rest(1)
            tclf(1)
            # whT last on sync: its 64-partition x 1-col layout DMAs slowly
            whT = constp.tile([64, 1], BF16, name="whT")
            nc.sync.dma_start(out=whT, in_=whT_d[:, :])
            head(0)
            zf_alloc(0)

            # ---------------- 3-deep interleaved schedule ----------------
            for i in range(BPC + 1):
                cur = i if i < BPC else None           # chain/softmax batch
                nxt = i + 1 if i + 1 < BPC else None   # gate batch
                prv = i - 1 if i >= 1 else None        # epilogue batch
                if i + 2 < BPC:
                    loads_gate(i + 2)
                    loads_rest(i + 2)
                # PE-dense warm block first: transposes, head mm, content_s mm
                if prv is not None:
                    wT_fin(prv)
                if nxt is not None:
                    head(nxt)
                    zf_alloc(nxt)
                if cur is not None:
                    content_s(cur)
                # coarse phase blocks: keep the PE stream dense per phase
                if nxt is not None:
                    for tcb in range(TC):
                        gate_pair(nxt, tcb)
                if cur is not None:
                    for h_ in range(2):
                        chain_pair(cur, h_)
                if prv is not None:
                    for tcb in range(TC):
                        epilogue_pair(prv, tcb)
                    out_dma(prv)
                if cur is not None:
                    softmax(cur)
                if i + 2 < BPC:
                    tclf(i + 2)
                if prv is not None:
                    del state[prv]
    return nc


_CACHED = {}


def _get_nc():
    if "nc" not in _CACHED:
        nc = bacc.Bacc("TRN2", target_bir_lowering=False)
        build(nc)
        nc.compile()
        _CACHED["nc"] = nc
    return _CACHED["nc"]


def _host_prep(inputs):
    import ml_dtypes
    bf = ml_dtypes.bfloat16
    f8 = ml_dtypes.float8_e4m3
    x = inputs["x"].astype(np.float32)
    h = inputs["hiddens"].astype(np.float32)
    c = inputs["cells"].astype(np.float32)
    V = inputs["V"].astype(np.float32)
    Wx, Wg, Ws, Wv, Wh = (inputs[k].astype(np.float32)
                          for k in ("Wx", "Wg", "Ws", "Wv", "Wh"))
    # DoubleRow layout: [p, i, r, t] with h = 256*i + 128*r + p
    dr = lambda a: np.ascontiguousarray(
        a.transpose(0, 2, 1).reshape(B, 4, 2, 128, T).transpose(0, 3, 1, 2, 4)
        .reshape(B, 128, 4096)).astype(f8)
    xT = dr(x)
    h8 = dr(h)
    hT = np.ascontiguousarray(h.transpose(0, 2, 1)).astype(bf)
    cT = np.ascontiguousarray(c.transpose(0, 2, 1)).astype(bf)
    vp = np.zeros((B, 64, H), np.float32); vp[:, :K] = V
    vp = vp.astype(bf)
    vT = np.zeros((B, H, 64), np.float32); vT[:, :, :K] = V.transpose(0, 2, 1)
    vT = vT.astype(bf)
    wxT = np.ascontiguousarray(
        (32.0 * Wx.T).reshape(4, 2, 128, H).transpose(2, 0, 1, 3)
        .reshape(128, 8192)).astype(f8)
    # wg8: fp8 DoubleRow layout [p, i, r, o], o: 0-48 = 32*Wg.T, 64-112 dup
    wgw = np.zeros((H, 128), np.float32)
    wgw[:, :K] = 32.0 * Wg.T
    wgw[:, 64:64 + K] = 32.0 * Wg.T
    wg8 = np.ascontiguousarray(
        wgw.reshape(4, 2, 128, 128).transpose(2, 0, 1, 3).reshape(128, 1024)
    ).astype(f8)
    w64 = lambda w: np.pad(np.ascontiguousarray(w.T), ((0, 0), (0, 64 - K))).astype(bf)
    wsT2, wvT = w64(0.5 * Ws), w64(Wv)
    whT = np.zeros((64, 1), np.float32); whT[:K, 0] = Wh[0]
    whT = whT.astype(bf)
    # wh_full / ecv base in the grouped (s, j) layout, j-width 49
    whf = np.zeros((128, CW), np.float32)
    ecvb = np.zeros((128, CW), np.float32)
    for col, s0, ns in PIECES:
        for s in range(ns):
            off = col + s * K
            whf[:, off:off + K] = Wh[0]
            for j in range(K):
                # 1/32 undoes the 32x in wg8 (g arrives as 32*g in gTx)
                ecvb[j, off + j] = 1.0 / 32.0
                ecvb[64 + j, off + j] = 1.0 / 32.0
    return {
        "xT": xT, "h8": h8, "hT": hT, "cT": cT, "vp": vp, "vT": vT,
        "wxT": wxT, "wg8": wg8, "wsT2": wsT2, "wvT": wvT, "whT": whT,
        "whf": whf.astype(bf), "ecvb": ecvb.astype(bf),
    }


def kernel(**inputs) -> np.ndarray:
    from concourse.bass_utils import run_bass_kernel_spmd

    nc = _get_nc()
    hp = _host_prep(inputs)
    shard_keys = ["xT", "h8", "hT", "cT", "vp", "vT"]
    rep_keys = ["wxT", "wg8", "wsT2", "wvT", "whT", "whf", "ecvb"]
    in_maps = []
    for i in range(NC_):
        m = {k: np.ascontiguousarray(hp[k][i * BPC:(i + 1) * BPC])
             for k in shard_keys}
        for k in rep_keys:
            m[k] = hp[k]
        in_maps.append(m)

    trace = bool(int(os.environ.get("KERNEL_TRACE", "0")))
    res = run_bass_kernel_spmd(nc, in_maps, core_ids=list(range(NC_)), trace=trace)
    _CACHED["exec_time_ns"] = res.exec_time_ns
    _CACHED["profile_json"] = getattr(res, "profile_json", None)
    outs = [np.asarray(res.results[i]["out"]).astype(np.float32).transpose(0, 2, 1)
            for i in range(NC_)]
    return np.ascontiguousarray(np.concatenate(outs, axis=0))


# revision 31
# speedup vs baseline: 1.0306x; 1.0306x over previous
"""Trainium2 Bass kernel for nn_AdaptiveBlock (B=64,T=512,H=1024,K=49).

Data-parallel over batch: 8 NeuronCores x 8 examples each, weights replicated.

Math (per example; h0=0 so the Whh term vanishes):
  th_g = tanh(0.5 * x @ Wx.T)            # sigmoid(z) = 0.5*(1+tanh(z/2))
  s2   = (1 + th_g) * tanh(cells)        # s = 0.5*s2
  g    = hiddens @ Wg.T                  # [T,49]
  cv   = V @ Wv.T                        # [49,49]
  z_t[t,k]  = sum_j Wh[j] tanh(cv[k,j] + g[t,j])
  z_ext[t]  = sum_j Wh[j] tanh((s2 @ (Ws/2).T + g)[t,j])
  e = exp([z_t, z_ext]); w = e/sum(e); beta' = 0.5*e_ext/sum(e)   # |z|<=6
  out = w[:, :49] @ V + beta' * s2 + hiddens

Host does all layout work (bf16/fp8 casts, transposes to [H,T], weight
transposes, selector/broadcast constants); device computes in [h-part,
t-free] layout and writes out^T [H,T] bf16, un-transposed on host.
Content chain uses the ecv selector-matmul trick: one matmul per psum
piece materializes cv[k,j]+g[t,j] for all (k,j). The chain runs as two
concurrent PE streams (row groups 0-63 / 64-127, duplicated gTx/ecv
rows) covering two t-chunks at once. Gate matmul runs fp8 DoubleRow.
Three-deep software pipeline interleaved at t-chunk grain:
epilogue(b-1) | chain+softmax(b) | gate(b+1).
"""

import os
import numpy as np

import concourse.bass as bass
import concourse.mybir as mybir
from concourse import bacc
import concourse.tile as tile
from concourse.masks import make_identity

FP32 = mybir.dt.float32
BF16 = mybir.dt.bfloat16
FP8 = mybir.dt.float8e4
AX = mybir.AxisListType
OP = mybir.AluOpType
AF = mybir.ActivationFunctionType

B, T, H, K = 64, 512, 1024, 49
NC_ = 8             # cores
BPC = B // NC_      # examples per core
TC = T // 128       # 4 t-chunks
HC = H // 128       # 8 h-chunks
# chain layout: three bank-aligned-enough pieces, segment = 49 cols (no pad)
PIECES = ((0, 0, 20), (980, 20, 20), (1960, 40, 9))  # (col, seg0, nsegs)
CW = 2401           # 49 segments x 49 j
CWE = 2402          # even tile width: keeps pool buffers 4B-aligned
MG = 114            # g-matmul output rows: 0-49 = g|ones, 64-113 = dup


def build(nc: bass.Bass):
    xT_d = nc.declare_dram_parameter("xT", [BPC, 128, 4096], FP8, isOutput=False)
    hT_d = nc.declare_dram_parameter("hT", [BPC, H, T], BF16, isOutput=False)
    cT_d = nc.declare_dram_parameter("cT", [BPC, H, T], BF16, isOutput=False)
    v_d = nc.declare_dram_parameter("vp", [BPC, 64, H], BF16, isOutput=False)
    vT_d = nc.declare_dram_parameter("vT", [BPC, H, 64], BF16, isOutput=False)
    wxT_d = nc.declare_dram_parameter("wxT", [128, 8192], FP8, isOutput=False)
    h8_d = nc.declare_dram_parameter("h8", [BPC, 128, 4096], FP8, isOutput=False)
    wgT_d = nc.declare_dram_parameter("wg8", [128, 1024], FP8, isOutput=False)
    wsT_d = nc.declare_dram_parameter("wsT2", [H, 64], BF16, isOutput=False)
    wvT_d = nc.declare_dram_parameter("wvT", [H, 64], BF16, isOutput=False)
    whT_d = nc.declare_dram_parameter("whT", [64, 1], BF16, isOutput=False)
    whf_d = nc.declare_dram_parameter("whf", [128, CW], BF16, isOutput=False)
    ecv_d = nc.declare_dram_parameter("ecvb", [128, CW], BF16, isOutput=False)
    out_d = nc.declare_dram_parameter("out", [BPC, H, T], BF16, isOutput=True)

    with tile.TileContext(nc) as tc:
        with (
            tc.tile_pool(name="const", bufs=1) as constp,
            tc.tile_pool(name="inp", bufs=2) as inp,
            tc.tile_pool(name="act", bufs=2) as actp,
            tc.tile_pool(name="sm", bufs=2) as smp,
            tc.tile_pool(name="outp", bufs=2) as outp,
            tc.tile_pool(name="psA", bufs=2, space="PSUM") as psA,
            tc.tile_pool(name="psB", bufs=2, space="PSUM") as psB,
            tc.tile_pool(name="psC", bufs=2, space="PSUM") as psC,
        ):
            # ---------------- constants / tiny memsets ----------
            ident = constp.tile([128, 128], BF16, name="ident")
            make_identity(nc, ident)
            ident_f = constp.tile([128, 128], FP32, name="ident_f")
            make_identity(nc, ident_f)
            ident32 = constp.tile([128, 128], BF16, name="ident32")
            nc.vector.tensor_scalar_mul(ident32, ident, 1.0 / 32.0)
            ones1 = constp.tile([1, 128], BF16, name="ones1")
            nc.gpsimd.memset(ones1, 1.0)
            onesT = constp.tile([1, T], BF16, name="onesT")
            nc.gpsimd.memset(onesT, 1.0)
            e49row = constp.tile([1, MG], BF16, name="e49row")
            nc.gpsimd.memset(e49row, 0.0)
            nc.gpsimd.memset(e49row[0:1, K:K + 1], 1.0)
            nc.gpsimd.memset(e49row[0:1, MG - 1:MG], 1.0)

            state = {}

            # ---------------- input loads (gate-critical first) ----------
            def loads_gate(b):
                xT = inp.tile([128, 4096], FP8, tag="xT", bufs=3, name=f"xT{b}")
                nc.sync.dma_start(out=xT, in_=xT_d[b])
                cT = inp.tile([128, HC * T], BF16, tag="cT", bufs=2, name=f"cT{b}")
                nc.gpsimd.dma_start(
                    out=cT.rearrange("p (c t) -> p c t", t=T),
                    in_=cT_d[b].rearrange("(c p) t -> p c t", p=128),
                )
                state[b] = {"xT": xT, "cT": cT}

            def loads_rest(b):
                st = state[b]
                hT = inp.tile([128, HC * T], BF16, tag="hT", bufs=4, name=f"hT{b}")
                nc.gpsimd.dma_start(
                    out=hT.rearrange("p (c t) -> p c t", t=T),
                    in_=hT_d[b].rearrange("(c p) t -> p c t", p=128),
                )
                h8 = inp.tile([128, 4096], FP8, tag="h8", bufs=3, name=f"h8{b}")
                nc.sync.dma_start(out=h8, in_=h8_d[b])
                st["h8"] = h8
                vn = inp.tile([64, H], BF16, tag="vn", bufs=4, name=f"vn{b}")
                nc.sync.dma_start(out=vn, in_=v_d[b])
                vT = inp.tile([128, HC * 64], BF16, tag="vT", bufs=3, name=f"vT{b}")
                nc.sync.dma_start(
                    out=vT.rearrange("p (c j) -> p c j", j=64),
                    in_=vT_d[b].rearrange("(c p) j -> p c j", p=128),
                )
                st.update({"hT": hT, "vn": vn, "vT": vT})

            # tanh(cells) for the whole example in one ACT instruction
            def tclf(b):
                st = state[b]
                tcl = actp.tile([128, HC * T], BF16, tag="tclf", bufs=2,
                                name=f"tclf{b}")
                nc.scalar.activation(tcl, st["cT"], AF.Tanh)
                st["tclf"] = tcl

            # ---------------- pipeline stage bodies ----------------
            def head(b):
                st = state[b]
                # cv = V @ Wv.T  -> scatter into ecv rows 49 and 113
                cvp = psA.tile([128, 512], FP32, tag="ps", name=f"cvp{b}")
                for kc in range(HC):
                    nc.tensor.matmul(
                        cvp[0:64, 0:64],
                        st["vT"][:, kc * 64:(kc + 1) * 64],
                        wvT[:, kc * 64:(kc + 1) * 64],
                        start=(kc == 0), stop=(kc == HC - 1),
                    )
                cv_sb = smp.tile([K, K], BF16, tag="cv", name=f"cv{b}")
                nc.vector.tensor_copy(cv_sb, cvp[0:K, 0:K])
                ecv_b = ecv[b % 2]
                for row in (K, 64 + K):
                    for col, s0, ns in PIECES:
                        nc.gpsimd.dma_start(
                            out=ecv_b[row:row + 1, col:col + ns * K].rearrange(
                                "p (s j) -> p s j", j=K),
                            in_=cv_sb[s0:s0 + ns],
                        )
                st["ecv"] = ecv_b
                # gT = 32*(hiddens @ Wg.T).T via fp8 DoubleRow;
                # rows 0-48 g, 49 ones, 64-112 g, 113 ones (1/32 in ecv/ident32)
                gp = psA.tile([128, 512], FP32, tag="ps", name=f"gp{b}")
                wg4 = wg8.rearrange("p (i r o) -> p i r o", r=2, o=128)
                h4 = st["h8"].rearrange("p (i r t) -> p i r t", r=2, t=T)
                for i in range(4):
                    nc.tensor.matmul(
                        gp[0:MG, 0:T],
                        wg4[:, i, :, 0:MG],
                        h4[:, i],
                        start=(i == 0), stop=False,
                        perf_mode=mybir.MatmulPerfMode.DoubleRow,
                    )
                nc.tensor.matmul(
                    gp[0:MG, 0:T], e49row, onesT, start=False, stop=True)
                gTx = smp.tile([128, T], BF16, tag="gTx", name=f"gTx{b}")
                nc.vector.tensor_copy(gTx[0:MG], gp[0:MG, 0:T])
                st["gTx"] = gTx

            def gate_pair(b, pr):
                # hc pair (2*pr, 2*pr+1): fp8 DoubleRow matmuls + tanh + s2
                st = state[b]
                if pr == 0:
                    st["s2"] = actp.tile([128, HC * T], BF16, tag="s2", bufs=3,
                                         name=f"s2{b}")
                wx4 = wxT.rearrange("p (i r o) -> p i r o", r=2, o=H)
                x4 = st["xT"].rearrange("p (i r t) -> p i r t", r=2, t=T)
                for i_ in range(2):
                    hc = 2 * pr + i_
                    pg = psA.tile([128, 512], FP32, tag="ps", name=f"pg{b}_{hc}")
                    for i in range(4):
                        nc.tensor.matmul(
                            pg,
                            wx4[:, i, :, hc * 128:(hc + 1) * 128],
                            x4[:, i],
                            start=(i == 0), stop=(i == 3),
                            perf_mode=mybir.MatmulPerfMode.DoubleRow,
                        )
                    tg = actp.tile([128, T], BF16, tag="thg", bufs=2,
                                   name=f"thg{b}_{hc}")
                    nc.scalar.activation(tg, pg, AF.Tanh, scale=1.0 / 64.0)
                    # s2 = (th_g + 1) * th_c
                    nc.vector.scalar_tensor_tensor(
                        out=st["s2"][:, hc * T:(hc + 1) * T],
                        in0=tg, scalar=1.0,
                        in1=st["tclf"][:, hc * T:(hc + 1) * T],
                        op0=OP.add, op1=OP.mult,
                    )

            def chain_pair(b, h_):
                # two t-chunks (2h, 2h+1) as concurrent PE row-group streams
                st = state[b]
                gTx, ecv_b = st["gTx"], st["ecv"]
                tcbs = (2 * h_, 2 * h_ + 1)
                lhs = (gTx[0:K + 1, tcbs[0] * 128:(tcbs[0] + 1) * 128],
                       gTx[64:64 + K + 1, tcbs[1] * 128:(tcbs[1] + 1) * 128])
                conts = []
                for s_ in range(2):
                    conts.append(smp.tile([128, CWE], BF16, tag="cont",
                                          name=f"cont{b}_{tcbs[s_]}"))
                pps = []
                for col, s0, ns in PIECES:
                    w_ = ns * K
                    pool_ = psC if ns == 9 else psB
                    pp = [pool_.tile([128, 1024 if w_ > 512 else 512], FP32,
                                     tag="pb", name=f"pp{b}_{tcbs[s_]}_{col}")
                          for s_ in range(2)]
                    off = 0
                    while off < w_:
                        cw = min(512, w_ - off)
                        for s_ in range(2):
                            rb = 64 * s_
                            nc.tensor.matmul(
                                pp[s_][:, off:off + cw],
                                lhs[s_],
                                ecv_b[rb:rb + K + 1, col + off:col + off + cw],
                                start=True, stop=True,
                            )
                        off += cw
                    pps.append((pp, col, w_))
                for pp, col, w_ in pps:
                    for s_ in range(2):
                        nc.scalar.activation(
                            conts[s_][:, col:col + w_], pp[s_][:, 0:w_], AF.Tanh)
                for s_ in range(2):
                    cw_t = smp.tile([128, CWE], BF16, tag="cw", bufs=1,
                                    name=f"cw{b}_{tcbs[s_]}")
                    nc.vector.tensor_mul(cw_t, conts[s_], wh_full)
                    nc.vector.tensor_reduce(
                        out=st["zf"][:, tcbs[s_] * 50:tcbs[s_] * 50 + K],
                        in_=cw_t[:, 0:CW].rearrange("p (s j) -> p s j", j=K),
                        axis=AX.X, op=OP.add,
                    )

            def content_s(b):
                st = state[b]
                csp = psA.tile([128, 512], FP32, tag="ps", name=f"csp{b}")
                for kc in range(HC):
                    nc.tensor.matmul(
                        csp[0:64, 0:T],
                        wsT[:, kc * 64:(kc + 1) * 64],
                        st["s2"][:, kc * T:(kc + 1) * T],
                        start=(kc == 0), stop=False,
                    )
                nc.tensor.matmul(
                    csp[0:64, 0:T], ident32[0:K + 1, 0:64], st["gTx"][0:K + 1],
                    start=False, stop=True,
                )
                tcs = smp.tile([K, T], BF16, tag="tcs", name=f"tcs{b}")
                nc.scalar.activation(tcs, csp[0:K, 0:T], AF.Tanh)
                zxp = psA.tile([128, 512], FP32, tag="ps", name=f"zxp{b}")
                for tcb in range(TC):
                    nc.tensor.matmul(
                        zxp[:, tcb:tcb + 1],
                        tcs[:, tcb * 128:(tcb + 1) * 128],
                        whT[0:K],
                        start=True, stop=True,
                        skip_group_check=True,
                    )
                nc.vector.tensor_copy(
                    st["zf"].rearrange("p (c j) -> p c j", j=50)[:, :, K:K + 1],
                    zxp[:, 0:TC])

            def softmax(b):
                # |z| <= sum|Wh| ~ 6 so exp cannot overflow: skip max-subtract
                st = state[b]
                ef = smp.tile([128, 4 * 50], FP32, tag="ef", name=f"ef{b}")
                nc.scalar.activation(ef, st["zf"], AF.Exp)
                den = smp.tile([128, 4], FP32, tag="den", name=f"dn{b}")
                nc.vector.tensor_reduce(
                    out=den, in_=ef.rearrange("p (c j) -> p c j", j=50),
                    axis=AX.X, op=OP.add)
                rec = smp.tile([128, 4], FP32, tag="rec", name=f"rc{b}")
                nc.vector.reciprocal(rec, den)
                brow0 = smp.tile([1, T], BF16, tag="br0", name=f"br0{b}")
                st["brow0"] = brow0
                st["wtp"] = []
                for tcb in range(TC):
                    wb = smp.tile([128, K], FP32, tag="wb", bufs=8, name=f"wb{b}_{tcb}")
                    nc.vector.tensor_scalar_mul(
                        wb, ef[:, tcb * 50:tcb * 50 + K], rec[:, tcb:tcb + 1])
                    bc = smp.tile([128, 1], FP32, tag="bc", bufs=4, name=f"bc{b}_{tcb}")
                    nc.vector.tensor_scalar(
                        out=bc, in0=ef[:, tcb * 50 + K:tcb * 50 + K + 1],
                        scalar1=rec[:, tcb:tcb + 1], scalar2=0.5,
                        op0=OP.mult, op1=OP.mult,
                    )
                    nc.gpsimd.dma_start(
                        out=brow0[0:1, tcb * 128:(tcb + 1) * 128], in_=bc)
                    st["wtp"].append(wb)

            def wT_fin(b):
                st = state[b]
                wTb = smp.tile([K, T], BF16, tag="wT", name=f"wT{b}")
                st["wT"] = wTb
                for tcb in range(TC):
                    wp = psC.tile([128, 512], FP32, tag="pb", name=f"wp{b}_{tcb}")
                    nc.tensor.transpose(
                        wp[0:K, 0:128], st["wtp"][tcb], ident_f)
                    nc.vector.tensor_copy(
                        wTb[:, tcb * 128:(tcb + 1) * 128], wp[0:K, 0:128])
                brp = psC.tile([128, 512], FP32, tag="pb", name=f"brp{b}")
                nc.tensor.matmul(brp, ones1, st["brow0"], start=True, stop=True)
                brow = smp.tile([128, T], BF16, tag="brow", name=f"brow{b}")
                nc.vector.tensor_copy(brow, brp)
                st["brow"] = brow
                st["ob"] = outp.tile([128, HC * T], BF16, tag="ob", name=f"ob{b}")

            def epilogue_pair(b, pr):
                st = state[b]
                cp = psB.tile([128, 1024], FP32, tag="pb", name=f"cp{b}_{pr}")
                ut = smp.tile([128, 1024], BF16, tag="ut", bufs=2, name=f"ut{b}_{pr}")
                nc.gpsimd.tensor_mul(
                    ut.rearrange("p (c t) -> p c t", t=T),
                    st["s2"][:, pr * 1024:(pr + 1) * 1024].rearrange(
                        "p (c t) -> p c t", t=T),
                    st["brow"][:, None, :].broadcast_to([128, 2, T]),
                )
                for i in range(2):
                    hc = 2 * pr + i
                    sl = cp[:, i * T:(i + 1) * T]
                    nc.tensor.matmul(
                        sl,
                        st["vn"][0:K, hc * 128:(hc + 1) * 128],
                        st["wT"],
                        start=True, stop=False,
                    )
                    nc.tensor.matmul(
                        sl, ident,
                        st["hT"][:, hc * T:(hc + 1) * T],
                        start=False, stop=False,
                    )
                    nc.tensor.matmul(
                        sl, ident, ut[:, i * T:(i + 1) * T],
                        start=False, stop=True,
                    )
                nc.scalar.copy(out=st["ob"][:, pr * 1024:(pr + 1) * 1024], in_=cp)

            def out_dma(b):
                st = state[b]
                eng = nc.sync if b % 2 == 0 else nc.scalar
                eng.dma_start(
                    out=out_d[b].rearrange("(c p) t -> p c t", p=128),
                    in_=st["ob"].rearrange("p (c t) -> p c t", t=T),
                )

            def zf_alloc(b):
                state[b]["zf"] = smp.tile([128, 4 * 50], FP32, tag="zf",
                                          name=f"zf{b}")

            # ---------------- prologue: gate(0) before anything else -------
            loads_gate(0)
            wxT = constp.tile([128, 8192], FP8, name="wxT")
            nc.sync.dma_start(out=wxT, in_=wxT_d[:, :])
            tclf(0)
            for pr in range(4):
                gate_pair(0, pr)

            # remaining constants on the scalar queue
            def load_w(dram, name, jw):
                t_ = constp.tile([128, HC * jw], BF16, name=name)
                nc.scalar.dma_start(
                    out=t_.rearrange("p (kc j) -> p kc j", j=jw),
                    in_=dram[:, :].rearrange("(kc p) j -> p kc j", p=128),
                )
                return t_
            wg8 = constp.tile([128, 1024], FP8, name="wg8")
            nc.scalar.dma_start(out=wg8, in_=wgT_d[:, :])
            wsT = load_w(wsT_d, "wsT", 64)
            wvT = load_w(wvT_d, "wvT", 64)
            wh_full = constp.tile([128, CWE], BF16, name="whf")
            nc.scalar.dma_start(out=wh_full[:, 0:CW], in_=whf_d[:, :])
            nc.gpsimd.memset(wh_full[:, CW:CWE], 0.0)
            ecv = []
            for v_ in range(2):
                e_ = constp.tile([128, CWE], BF16, name=f"ecv{v_}")
                nc.scalar.dma_start(out=e_[:, 0:CW], in_=ecv_d[:, :])
                ecv.append(e_)

            loads_rest(0)
            loads_gate(1)
            loads_rest(1)
            tclf(1)
            # whT last on sync: its 64-partition x 1-col layout DMAs slowly
            whT = constp.tile([64, 1], BF16, name="whT")
            nc.sync.dma_start(out=whT, in_=whT_d[:, :])
            head(0)
            zf_alloc(0)

            # ---------------- 3-deep interleaved schedule ----------------
            for i in range(BPC + 1):
                cur = i if i < BPC else None           # chain/softmax batch
                nxt = i + 1 if i + 1 < BPC else None   # gate batch
                prv = i - 1 if i >= 1 else None        # epilogue batch
                if i + 2 < BPC:
                    loads_gate(i + 2)
                    loads_rest(i + 2)
                if prv is not None:
                    wT_fin(prv)
                if nxt is not None:
                    head(nxt)
                    zf_alloc(nxt)
                if cur is not None:
                    content_s(cur)
                for tcb in range(TC):
                    if nxt is not None:
                        gate_pair(nxt, tcb)
                    if cur is not None and tcb < 2:
                        chain_pair(cur, tcb)
                    if prv is not None:
                        epilogue_pair(prv, tcb)
                if prv is not None:
                    out_dma(prv)
                if cur is not None:
                    softmax(cur)
                if i + 2 < BPC:
                    tclf(i + 2)
                if prv is not None:
                    del state[prv]
    return nc


_CACHED = {}


def _get_nc():
    if "nc" not in _CACHED:
        nc = bacc.Bacc("TRN2", target_bir_lowering=False)
        build(nc)
        nc.compile()
        _CACHED["nc"] = nc
    return _CACHED["nc"]


def _host_prep(inputs):
    import ml_dtypes
    bf = ml_dtypes.bfloat16
    f8 = ml_dtypes.float8_e4m3
    x = inputs["x"].astype(np.float32)
    h = inputs["hiddens"].astype(np.float32)
    c = inputs["cells"].astype(np.float32)
    V = inputs["V"].astype(np.float32)
    Wx, Wg, Ws, Wv, Wh = (inputs[k].astype(np.float32)
                          for k in ("Wx", "Wg", "Ws", "Wv", "Wh"))
    # DoubleRow layout: [p, i, r, t] with h = 256*i + 128*r + p
    dr = lambda a: np.ascontiguousarray(
        a.transpose(0, 2, 1).reshape(B, 4, 2, 128, T).transpose(0, 3, 1, 2, 4)
        .reshape(B, 128, 4096)).astype(f8)
    xT = dr(x)
    h8 = dr(h)
    hT = np.ascontiguousarray(h.transpose(0, 2, 1)).astype(bf)
    cT = np.ascontiguousarray(c.transpose(0, 2, 1)).astype(bf)
    vp = np.zeros((B, 64, H), np.float32); vp[:, :K] = V
    vp = vp.astype(bf)
    vT = np.zeros((B, H, 64), np.float32); vT[:, :, :K] = V.transpose(0, 2, 1)
    vT = vT.astype(bf)
    wxT = np.ascontiguousarray(
        (32.0 * Wx.T).reshape(4, 2, 128, H).transpose(2, 0, 1, 3)
        .reshape(128, 8192)).astype(f8)
    # wg8: fp8 DoubleRow layout [p, i, r, o], o: 0-48 = 32*Wg.T, 64-112 dup
    wgw = np.zeros((H, 128), np.float32)
    wgw[:, :K] = 32.0 * Wg.T
    wgw[:, 64:64 + K] = 32.0 * Wg.T
    wg8 = np.ascontiguousarray(
        wgw.reshape(4, 2, 128, 128).transpose(2, 0, 1, 3).reshape(128, 1024)
    ).astype(f8)
    w64 = lambda w: np.pad(np.ascontiguousarray(w.T), ((0, 0), (0, 64 - K))).astype(bf)
    wsT2, wvT = w64(0.5 * Ws), w64(Wv)
    whT = np.zeros((64, 1), np.float32); whT[:K, 0] = Wh[0]
    whT = whT.astype(bf)
    # wh_full / ecv base in the grouped (s, j) layout, j-width 49
    whf = np.zeros((128, CW), np.float32)
    ecvb = np.zeros((128, CW), np.float32)
    for col, s0, ns in PIECES:
        for s in range(ns):
            off = col + s * K
            whf[:, off:off + K] = Wh[0]
            for j in range(K):
                # 1/32 undoes the 32x in wg8 (g arrives as 32*g in gTx)
                ecvb[j, off + j] = 1.0 / 32.0
                ecvb[64 + j, off + j] = 1.0 / 32.0
    return {
        "xT": xT, "h8": h8, "hT": hT, "cT": cT, "vp": vp, "vT": vT,
        "wxT": wxT, "wg8": wg8, "wsT2": wsT2, "wvT": wvT, "whT": whT,
        "whf": whf.astype(bf), "ecvb": ecvb.astype(bf),
    }


def kernel(**inputs) -> np.ndarray:
    from concourse.bass_utils import run_bass_kernel_spmd

    nc = _get_nc()
    hp = _host_prep(inputs)
    shard_keys = ["xT", "h8", "hT", "cT", "vp", "vT"]
    rep_keys = ["wxT", "wg8", "wsT2", "wvT", "whT", "whf", "ecvb"]
    in_maps = []
    for i in range(NC_):
        m = {k: np.ascontiguousarray(hp[k][i * BPC:(i + 1) * BPC])
             for k in shard_keys}
        for k in rep_keys:
            m[k] = hp[k]
        in_maps.append(m)

    trace = bool(int(os.environ.get("KERNEL_TRACE", "0")))
    res = run_bass_kernel_spmd(nc, in_maps, core_ids=list(range(NC_)), trace=trace)
    _CACHED["exec_time_ns"] = res.exec_time_ns
    _CACHED["profile_json"] = getattr(res, "profile_json", None)
    outs = [np.asarray(res.results[i]["out"]).astype(np.float32).transpose(0, 2, 1)
            for i in range(NC_)]
    return np.ascontiguousarray(np.concatenate(outs, axis=0))
